# revision 1
# baseline (speedup 1.0000x reference)
"""Trainium2 Bass kernel for a dual cross-attention block.

Computes, per batch element b (8 total, one per NeuronCore):
    Q  = obj @ Wq.T + bq                    [2048, 1024]
    Kx = x @ Wxk.T + bxk,  Vx = x @ Wxv.T + bxv   for x in {sub, scene}
    Ix = LayerNorm(obj + softmax(Q Kx.T / 32) Vx)  -> (I1, I2)

Design:
  - data-parallel over batch: core c handles batch element c (no collectives)
  - host side does layout only (transposes); all FLOPs on device
  - projections run as float32r matmuls (full PE speed, fp32-ish precision)
  - attention operands (Q_T, K_T, V, exp(S_T)) are bf16; accumulation fp32
  - scores are computed transposed (S_T[k,q]) so softmax'd weights are
    directly usable as the stationary operand of the PV matmul (no on-chip
    transposes anywhere); softmax max-subtraction is skipped (scores are
    bounded ~|2.5|) and the denominator comes from N=1 matmuls against ones
  - residual + layernorm fused on DVE/ACT: scalar_tensor_tensor computes
    (O*recip_denom)+obj and its row-sum in one pass; Square+accum gives the
    second moment; Identity activation applies (x-mu)*rstd
"""

import os
import numpy as np

SQ = 2048
SKV = 1024
EMB = 1024
PROJ = 1024
NCORES = 8
EPS = 1e-5
SCALE = PROJ ** -0.5

_CACHE = {}
LAST_RESULTS = None


def _build():
    import concourse.bass as bass
    import concourse.tile as tile
    import concourse.mybir as mybir
    from concourse import bacc

    dt = mybir.dt
    f32 = dt.float32
    f32r = dt.float32r
    bf16 = dt.bfloat16
    Act = mybir.ActivationFunctionType
    Alu = mybir.AluOpType

    nc = bacc.Bacc("TRN2", debug=False)

    # ---- DRAM I/O ----
    objT_d = nc.dram_tensor("objT", [EMB, SQ], f32r, kind="ExternalInput")
    obj_d = nc.dram_tensor("obj_nat", [SQ, EMB], f32, kind="ExternalInput")
    subT_d = nc.dram_tensor("subT", [EMB, SKV], f32r, kind="ExternalInput")
    scnT_d = nc.dram_tensor("scnT", [EMB, SKV], f32r, kind="ExternalInput")
    w_d = {
        n: nc.dram_tensor(f"W{n}T", [EMB, PROJ], f32r, kind="ExternalInput")
        for n in ["q", "sk", "sv", "ek", "ev"]
    }
    bq_d = nc.dram_tensor("bq", [128, 8], f32, kind="ExternalInput")
    bsk_d = nc.dram_tensor("bsk", [128, 8], f32, kind="ExternalInput")
    bek_d = nc.dram_tensor("bek", [128, 8], f32, kind="ExternalInput")
    bsvT_d = nc.dram_tensor("bsvT", [1, PROJ], f32r, kind="ExternalInput")
    bevT_d = nc.dram_tensor("bevT", [1, PROJ], f32r, kind="ExternalInput")
    ones_d = nc.dram_tensor("ones_row", [1, 128], f32r, kind="ExternalInput")
    G_d = nc.dram_tensor("G", [128, EMB], f32, kind="ExternalInput")
    B_d = nc.dram_tensor("Bb", [128, EMB], f32, kind="ExternalInput")
    I1_d = nc.dram_tensor("I1", [SQ, EMB], f32, kind="ExternalOutput")
    I2_d = nc.dram_tensor("I2", [SQ, EMB], f32, kind="ExternalOutput")

    EC = EMB // 128   # contraction chunks
    PC = PROJ // 128  # p chunks

    with tile.TileContext(nc) as tc:
        with (
            tc.tile_pool(name="const", bufs=1) as cpool,
            tc.tile_pool(name="kv", bufs=1) as kv,
        ):
            ones_col = cpool.tile([128, 1], bf16, name="ones_col")
            nc.vector.memset(ones_col, 1.0)
            ones_row = cpool.tile([1, 128], f32r, name="ones_row")
            nc.sync.dma_start(ones_row, ones_d[:, :])
            eps_s = cpool.tile([128, 1], f32, name="eps_s")
            nc.vector.memset(eps_s, EPS)
            bq_s = cpool.tile([128, 8], f32, name="bq_s")
            nc.sync.dma_start(bq_s, bq_d[:, :])
            bsk_s = cpool.tile([128, 8], f32, name="bsk_s")
            nc.sync.dma_start(bsk_s, bsk_d[:, :])
            bek_s = cpool.tile([128, 8], f32, name="bek_s")
            nc.sync.dma_start(bek_s, bek_d[:, :])

            def load_rows(pool, dram, tag, ncols, eng=None, flip=False,
                          engs=None):
                ts = []
                for ec in range(EC):
                    t = pool.tile([128, ncols], f32r, tag=tag, bufs=8,
                                  name=f"{tag}{ec}")
                    if engs is not None:
                        e = engs[ec]
                    else:
                        e = eng or (nc.gpsimd if (ec % 2) ^ flip else nc.sync)
                    e.dma_start(t, dram[ec * 128:(ec + 1) * 128, :])
                    ts.append(t)
                return ts

            # ---------- K/V projections (float32r) ----------
            def proj_K(src, wts, bias, tag):
                """K_T[p,k] tiles: 8 x [128(p-chunk), SKV] bf16."""
                out = []
                for pc in range(PC):
                    kt = kv.tile([128, SKV], bf16, tag="K", bufs=16,
                                 name=f"{tag}{pc}")
                    out.append(kt)
                for pc in range(PC):
                    for kn in range(SKV // 512):
                        ps = pp.tile([128, 512], f32, tag="pp", name="ps_k")
                        for ec in range(EC):
                            nc.tensor.matmul(
                                ps,
                                wts[ec][:, pc * 128:(pc + 1) * 128],
                                src[ec][:, kn * 512:(kn + 1) * 512],
                                start=(ec == 0), stop=(ec == EC - 1),
                            )
                        nc.vector.tensor_scalar_add(
                            out[pc][:, kn * 512:(kn + 1) * 512], ps,
                            bias[:, pc:pc + 1])
                return out

            def proj_V(src, wts, biasT, tag):
                """V[k,p] tiles: 8 x [128(k-chunk), PROJ] bf16."""
                out = []
                for kc in range(SKV // 128):
                    vt = kv.tile([128, PROJ], bf16, tag="V", bufs=16,
                                 name=f"{tag}{kc}")
                    out.append(vt)
                for kc in range(SKV // 128):
                    for po in range(PROJ // 512):
                        ps = pp.tile([128, 512], f32, tag="pp", name="ps_v")
                        for ec in range(EC):
                            nc.tensor.matmul(
                                ps,
                                src[ec][:, kc * 128:(kc + 1) * 128],
                                wts[ec][:, po * 512:(po + 1) * 512],
                                start=(ec == 0), stop=False,
                            )
                        nc.tensor.matmul(
                            ps,
                            ones_row[:1, :],
                            biasT[:1, po * 512:(po + 1) * 512],
                            start=False, stop=True,
                        )
                        nc.vector.tensor_copy(out[kc][:, po * 512:(po + 1) * 512], ps)
                return out

            qt = [kv.tile([128, SQ], bf16, tag="QT", bufs=8, name=f"qt{pc}")
                  for pc in range(PC)]
            with (
                tc.tile_pool(name="pp", bufs=4, space="PSUM") as pp,
                tc.tile_pool(name="acts", bufs=1) as acts,
            ):
                # first phase: fan the 16 startup tiles across 4 queues
                q4 = [nc.sync, nc.gpsimd, nc.scalar]
                sub_t = load_rows(acts, subT_d, "srcT", SKV,
                                  engs=[q4[ec % 3] for ec in range(EC)])
                # rotating weight pools: next weight prefetches during the
                # current projection's matmuls
                wpools = [tc.alloc_tile_pool(name="w0", bufs=1),
                          tc.alloc_tile_pool(name="w1", bufs=1, side="right")]
                wsk = load_rows(wpools[0], w_d["sk"], "wsk", PROJ,
                                engs=[q4[(ec + 1) % 3] for ec in range(EC)])
                wsv = load_rows(wpools[1], w_d["sv"], "wsv", PROJ)
                bsvT_s = wpools[1].tile([1, PROJ], f32r, name="bsvT_s")
                nc.sync.dma_start(bsvT_s, bsvT_d[:, :])
                skt = proj_K(sub_t, wsk, bsk_s, "skt")
                wpools[0].release()
                wpools.append(tc.alloc_tile_pool(name="w2", bufs=1))  # left
                wek = load_rows(wpools[2], w_d["ek"], "wek", PROJ)
                scn_t = load_rows(acts, scnT_d, "srcT", SKV, flip=True)
                svt = proj_V(sub_t, wsv, bsvT_s, "svt")
                wpools[1].release()
                wpools.append(tc.alloc_tile_pool(name="w3", bufs=1, side="right"))
                wev = load_rows(wpools[3], w_d["ev"], "wev", PROJ)
                bevT_s = wpools[3].tile([1, PROJ], f32r, name="bevT_s")
                nc.sync.dma_start(bevT_s, bevT_d[:, :])
                ekt = proj_K(scn_t, wek, bek_s, "ekt")
                wpools[2].release()
                wpools.append(tc.alloc_tile_pool(name="w4", bufs=1))  # left
                wq = load_rows(wpools[4], w_d["q"], "wq", PROJ)
                evt = proj_V(scn_t, wev, bevT_s, "evt")
                wpools[3].release()

                # ---- Q projection (float32r) -> bf16 Q_T ----
                # objT streamed as [128, 512] quarters on two DMA queues
                otp = tc.alloc_tile_pool(name="otp", bufs=1, side="right")
                for sq4 in range(4):
                    ot = []
                    for ec in range(EC):
                        t = otp.tile([128, 512], f32r, tag="ot", bufs=12,
                                     name=f"ot{sq4}_{ec}")
                        eng = nc.gpsimd if ec % 2 else nc.sync
                        eng.dma_start(
                            t, objT_d[ec * 128:(ec + 1) * 128,
                                      sq4 * 512:(sq4 + 1) * 512])
                        ot.append(t)
                    for pc in range(PC):
                        ps = pp.tile([128, 512], f32, tag="pp", name="ps_q")
                        for ec in range(EC):
                            nc.tensor.matmul(
                                ps,
                                wq[ec][:, pc * 128:(pc + 1) * 128],
                                ot[ec][:, :],
                                start=(ec == 0), stop=(ec == EC - 1),
                            )
                        nc.vector.tensor_scalar_add(
                            qt[pc][:, sq4 * 512:(sq4 + 1) * 512],
                            ps, bq_s[:, pc:pc + 1])
                otp.release()
                wpools[4].release()

            # ---------- attentions ----------
            with (
                tc.tile_pool(name="spp", bufs=1, space="PSUM") as spp,
                tc.tile_pool(name="opp", bufs=1, space="PSUM") as opp,
                tc.tile_pool(name="et", bufs=1) as etp,
                tc.tile_pool(name="epi", bufs=1) as epi,
                tc.tile_pool(name="small", bufs=1) as smp,
            ):
                G_s = epi.tile([128, EMB], f32, name="G_s")
                nc.sync.dma_start(G_s, G_d[:, :])
                B_s = epi.tile([128, EMB], f32, name="B_s")
                nc.sync.dma_start(B_s, B_d[:, :])
                def attention(K, V, out_d, aname):
                    for qc in range(SQ // 512):
                        et = []
                        for kc in range(SKV // 128):
                            ps = spp.tile([128, 512], f32, tag="sps", bufs=2,
                                          name=f"sps_{aname}")
                            for pc in range(PC):
                                nc.tensor.matmul(
                                    ps,
                                    K[pc][:, kc * 128:(kc + 1) * 128],
                                    qt[pc][:, qc * 512:(qc + 1) * 512],
                                    start=(pc == 0), stop=(pc == PC - 1),
                                )
                            e = etp.tile([128, 512], bf16, tag="et", bufs=20,
                                         name=f"et_{aname}{kc}")
                            nc.scalar.activation(e, ps, Act.Exp, scale=SCALE)
                            et.append(e)
                        for qs in range(4):
                            q0 = qc * 512 + qs * 128
                            ops = opp.tile([128, EMB], f32, tag="ops", bufs=2,
                                           name=f"ops_{aname}")
                            dps = spp.tile([128, 1], f32, tag="den", bufs=2,
                                           name=f"den_{aname}")
                            # kc outer: one stationary load feeds 2 PV halves
                            # + the denominator column
                            klast = SKV // 128 - 1
                            for kc in range(SKV // 128):
                                stat = et[kc][:, qs * 128:(qs + 1) * 128]
                                for po in range(PROJ // 512):
                                    nc.tensor.matmul(
                                        ops[:, po * 512:(po + 1) * 512],
                                        stat,
                                        V[kc][:, po * 512:(po + 1) * 512],
                                        start=(kc == 0), stop=(kc == klast),
                                    )
                                nc.tensor.matmul(
                                    dps, stat, ones_col[:, :],
                                    start=(kc == 0), stop=(kc == klast),
                                )
                            rcp = smp.tile([128, 1], f32, tag="sm", bufs=32,
                                           name="rcp")
                            nc.vector.reciprocal(rcp, dps)
                            ob = epi.tile([128, EMB], f32, tag="ob", bufs=4,
                                          name="ob")
                            nc.sync.dma_start(ob, obj_d[q0:q0 + 128, :])
                            x = epi.tile([128, EMB], f32, tag="x", bufs=4,
                                         name="x")
                            mus = smp.tile([128, 1], f32, tag="sm", bufs=32,
                                           name="mus")
                            nc.vector.scalar_tensor_tensor(
                                x, ops, rcp, ob, op0=Alu.mult, op1=Alu.add,
                                accum_out=mus)
                            sq = epi.tile([128, EMB], f32, tag="sq", bufs=2,
                                          name="sq")
                            ssq = smp.tile([128, 1], f32, tag="sm", bufs=32,
                                           name="ssq")
                            nc.scalar.activation(sq, x, Act.Square,
                                                 accum_out=ssq)
                            mu = smp.tile([128, 1], f32, tag="sm", bufs=32,
                                          name="mu")
                            nc.vector.tensor_scalar_mul(mu, mus, 1.0 / EMB)
                            msq = smp.tile([128, 1], f32, tag="sm", bufs=32,
                                           name="msq")
                            nc.vector.tensor_mul(msq, mu, mu)
                            var = smp.tile([128, 1], f32, tag="sm", bufs=32,
                                           name="var")
                            nc.vector.scalar_tensor_tensor(
                                var, ssq, 1.0 / EMB, msq,
                                op0=Alu.mult, op1=Alu.subtract)
                            sd = smp.tile([128, 1], f32, tag="sm", bufs=32,
                                          name="sd")
                            nc.scalar.activation(sd, var, Act.Sqrt, bias=eps_s)
                            rstd = smp.tile([128, 1], f32, tag="sm", bufs=32,
                                            name="rstd")
                            nc.vector.reciprocal(rstd, sd)
                            nmr = smp.tile([128, 1], f32, tag="sm", bufs=32,
                                           name="nmr")
                            nc.vector.scalar_tensor_tensor(
                                nmr, mu, -1.0, rstd, op0=Alu.mult, op1=Alu.mult)
                            t = epi.tile([128, EMB], f32, tag="t", bufs=2,
                                         name="t")
                            nc.scalar.activation(t, x, Act.Identity,
                                                 bias=nmr, scale=rstd)
                            o = epi.tile([128, EMB], f32, tag="o", bufs=4,
                                         name="o")
                            nc.vector.tensor_mul(o, t, G_s)
                            nc.vector.tensor_add(o, o, B_s)
                            nc.sync.dma_start(out_d[q0:q0 + 128, :], o)

                attention(skt, svt, I1_d, "s")
                attention(ekt, evt, I2_d, "e")

    nc.compile()
    return nc


def _prep_in_maps(inputs):
    f = lambda a: np.ascontiguousarray(np.asarray(a, dtype=np.float32))
    obj = f(inputs["obj"])
    sub = f(inputs["sub"])
    scene = f(inputs["scene"])
    shared = {}
    for n in ["q", "sk", "sv", "ek", "ev"]:
        shared[f"W{n}T"] = f(np.asarray(inputs[f"W_{n}"]).T)
    for key, n in [("bq", "q"), ("bsk", "sk"), ("bek", "ek")]:
        shared[key] = f(np.asarray(inputs[f"b_{n}"]).reshape(8, 128).T)
    shared["bsvT"] = f(np.asarray(inputs["b_sv"]).reshape(1, PROJ))
    shared["bevT"] = f(np.asarray(inputs["b_ev"]).reshape(1, PROJ))
    shared["ones_row"] = np.ones((1, 128), np.float32)
    shared["G"] = f(np.broadcast_to(np.asarray(inputs["ln_g"]), (128, EMB)))
    shared["Bb"] = f(np.broadcast_to(np.asarray(inputs["ln_b"]), (128, EMB)))
    in_maps = []
    for b in range(NCORES):
        m = dict(shared)
        m["objT"] = f(obj[b].T)
        m["obj_nat"] = obj[b]
        m["subT"] = f(sub[b].T)
        m["scnT"] = f(scene[b].T)
        in_maps.append(m)
    return in_maps


def kernel(**inputs):
    global LAST_RESULTS
    from concourse import bass_utils

    if "nc" not in _CACHE:
        _CACHE["nc"] = _build()
    nc = _CACHE["nc"]
    in_maps = _prep_in_maps(inputs)
    res = bass_utils.run_bass_kernel_spmd(
        nc, in_maps, core_ids=list(range(NCORES)))
    LAST_RESULTS = res
    I1 = np.stack([res.results[c]["I1"] for c in range(NCORES)])
    I2 = np.stack([res.results[c]["I2"] for c in range(NCORES)])
    return I1, I2



# revision 47
# speedup vs baseline: 2.5363x; 2.5363x over previous
"""Trainium2 Bass kernel for a dual cross-attention block.

Computes, per batch element b (8 total, one per NeuronCore):
    Q  = obj @ Wq.T + bq                    [2048, 1024]
    Kx = x @ Wxk.T + bxk,  Vx = x @ Wxv.T + bxv   for x in {sub, scene}
    Ix = LayerNorm(obj + softmax(Q Kx.T / 32) Vx)  -> (I1, I2)

Design:
  - data-parallel over batch: core c handles batch element c (no collectives)
  - all matmuls run as fp8(e4m3) DoubleRow (2 contraction rows per PE pass);
    activations/weights are quantized host-side (weights pre-scaled x64 for
    fp8 range), Q/K stored on-chip at 4x scale, V at 1x; the scale factors
    fold into the exp() scale and the PSUM->SBUF conversions
  - softmax denominator d comes from an fp8 ones-column matmul sharing the
    PV stationary operand; instead of dividing by d, the residual is scaled:
    LN(obj + O/d) == LN(d*obj + O) with the eps correction applied as
    eps*d^2 (exact identity; LN is scale-invariant per row)
  - V-projection bias is folded into the obj residual uploads (rank-1
    identity: P@(V + 1 b^T) = P@V + d b^T, and d*obj + d*b^T = d*(obj+b^T))
  - LN stats via one DVE bn_stats pass; rstd = exp(-0.5*ln(var + eps*d^2));
    an explicit act-table load of the set containing {Exp, Ln, Identity,
    Square} pins ONE table for the whole kernel (no reload churn)
  - the five projections, Q-chunk projections, scores, PV and the LN
    epilogues are software-pipelined in one woven emission order so PE,
    ACT, DVE and Pool all stay busy from ~15us onward
"""

import numpy as np
import ml_dtypes

SQ = 2048
SKV = 1024
EMB = 1024
PROJ = 1024
NCORES = 8
EPS = 1e-5
SCALE = PROJ ** -0.5

WS = 64.0   # weight upload scale (fp8 range)
QS = 4.0    # Q/K on-chip storage scale
EC = EMB // 128   # 8 contraction chunks of 128
NPAIR = EC // 2   # 4 DoubleRow pairs

_CACHE = {}
LAST_RESULTS = None

import os as _os
K1DVE = _os.environ.get("K1DVE", "0") == "1"   # K1 convs on DVE (else ACT)
K2DVE = _os.environ.get("K2DVE", "1") == "1"   # K2 convs on DVE (else ACT)
QDVE = int(_os.environ.get("QDVE", "1"))       # of 2 Q convs per pair on DVE
BNMOD = int(_os.environ.get("BNMOD", "2"))     # bn_stats when c % BNMOD == 0
NMRACT = _os.environ.get("NMRACT", "1") == "1"


def _build(skip_gb=False):
    import concourse.bass as bass
    import concourse.tile as tile
    import concourse.mybir as mybir
    from concourse import bacc
    from concourse.hw_specs import get_activation_tables

    dt = mybir.dt
    f32 = dt.float32
    bf16 = dt.bfloat16
    f8 = dt.float8e4
    Act = mybir.ActivationFunctionType
    Alu = mybir.AluOpType
    DR = mybir.MatmulPerfMode.DoubleRow

    nc = bacc.Bacc("TRN2", debug=False)

    # one activation table covering every function we use
    tables = list(get_activation_tables(nc.m.arch).items())
    need = {Act.Exp, Act.Ln, Act.Identity, Act.Square}
    act_set_id = next(i for i, (_, s) in enumerate(tables) if need <= s)

    # ---- DRAM I/O ----
    objT_d = nc.dram_tensor("objT", [SQ // 512, 128, EC, 512], f8,
                            kind="ExternalInput")
    subT_d = nc.dram_tensor("subT", [SKV // 512, 128, EC, 512], f8,
                            kind="ExternalInput")
    scnT_d = nc.dram_tensor("scnT", [SKV // 512, 128, EC, 512], f8,
                            kind="ExternalInput")
    w_d = {
        n: nc.dram_tensor(f"W{n}T", [NPAIR, 128, 2, PROJ], f8,
                          kind="ExternalInput")
        for n in ["q", "sk", "ek", "sv", "ev"]
    }
    objr1_d = nc.dram_tensor("objr1", [SQ // 128, 128, EMB], bf16,
                             kind="ExternalInput")
    objr2_d = nc.dram_tensor("objr2", [SQ // 128, 128, EMB], bf16,
                             kind="ExternalInput")
    # per-row sums of objr1/objr2 (column j = row block j) and the identity
    objr1s_d = nc.dram_tensor("objr1s", [128, SQ // 128], f32,
                              kind="ExternalInput")
    objr2s_d = nc.dram_tensor("objr2s", [128, SQ // 128], f32,
                              kind="ExternalInput")
    ident_d = nc.dram_tensor("ident", [128, 128], bf16,
                             kind="ExternalInput")
    bq_d = nc.dram_tensor("bq", [128, EC], f32, kind="ExternalInput")
    bsk_d = nc.dram_tensor("bsk", [128, EC], f32, kind="ExternalInput")
    bek_d = nc.dram_tensor("bek", [128, EC], f32, kind="ExternalInput")
    G_d = nc.dram_tensor("G", [128, EMB], bf16, kind="ExternalInput")
    B_d = nc.dram_tensor("Bb", [128, EMB], bf16, kind="ExternalInput")
    I1_d = nc.dram_tensor("I1", [SQ // 128, 128, EMB], bf16,
                          kind="ExternalOutput")
    I2_d = nc.dram_tensor("I2", [SQ // 128, 128, EMB], bf16,
                          kind="ExternalOutput")

    with tile.TileContext(nc) as tc:
        with (
            tc.tile_pool(name="const", bufs=1) as cpool,
            tc.tile_pool(name="kv", bufs=1) as kv,
            tc.tile_pool(name="wp", bufs=1) as wp,
            tc.tile_pool(name="src", bufs=1) as srcp,
            tc.tile_pool(name="etp", bufs=1) as etp,
            tc.tile_pool(name="epi", bufs=1) as epi,
            tc.tile_pool(name="smp", bufs=1) as smp,
            tc.tile_pool(name="pp", bufs=1, space="PSUM") as pp,
            tc.tile_pool(name="opp", bufs=1, space="PSUM") as opp,
            tc.tile_pool(name="dpp", bufs=1, space="PSUM") as dpp,
        ):
            nc.scalar.add_instruction(mybir.InstLoadActFuncSet(
                name=nc.get_next_instruction_name(),
                act_func_set_id=act_set_id, ins=[], outs=[]))

            ones2 = cpool.tile([128, 2, 16], f8, name="ones2")
            nc.vector.memset(ones2, 1.0)
            I_s = cpool.tile([128, 128], bf16, name="I_s")
            nc.scalar.dma_start(I_s, ident_d[:, :])
            so1_s = cpool.tile([128, SQ // 128], f32, name="so1_s")
            nc.scalar.dma_start(so1_s, objr1s_d[:, :])
            so2_s = cpool.tile([128, SQ // 128], f32, name="so2_s")
            nc.scalar.dma_start(so2_s, objr2s_d[:, :])
            # [ones | V-rowsum] fp8 moving operand for the d/SumO matmul
            vsum1 = cpool.tile([128, EC, 2], f8, name="vsum1")
            nc.vector.memset(vsum1, 1.0)
            vsum2 = cpool.tile([128, EC, 2], f8, name="vsum2")
            nc.vector.memset(vsum2, 1.0)
            bq_s = cpool.tile([128, EC], f32, name="bq_s")
            nc.scalar.dma_start(bq_s, bq_d[:, :])
            bsk_s = cpool.tile([128, EC], f32, name="bsk_s")
            nc.scalar.dma_start(bsk_s, bsk_d[:, :])
            bek_s = cpool.tile([128, EC], f32, name="bek_s")
            nc.scalar.dma_start(bek_s, bek_d[:, :])
            G_s = cpool.tile([128, EMB], bf16, name="G_s")
            nc.scalar.dma_start(G_s, G_d[:, :])
            B_s = cpool.tile([128, EMB], bf16, name="B_s")
            nc.scalar.dma_start(B_s, B_d[:, :])

            # resident fp8 operands for the attention phase
            kt1 = kv.tile([128, EC, SKV], f8, name="kt1")
            kt2 = kv.tile([128, EC, SKV], f8, name="kt2")
            vt1 = kv.tile([128, EC, PROJ], f8, name="vt1")
            vt2 = kv.tile([128, EC, PROJ], f8, name="vt2")
            qt = kv.tile([128, EC, SQ], f8, name="qt")

            # weight chunks; DMA issue order tracks consumption order so the
            # serialized DMA engines deliver operands just in time
            wch = {}
            qi = [0]

            def load_w(n):
                wch[n] = []
                for i in range(NPAIR):
                    t = wp.tile([128, 2, PROJ], f8, tag="w", bufs=20,
                                name=f"w{n}{i}")
                    eng = (nc.sync, nc.gpsimd)[qi[0] % 2]
                    qi[0] += 1
                    eng.dma_start(t, w_d[n][i])
                    wch[n].append(t)

            def load_src(dram, tag, q0):
                ts = []
                for c in range(SKV // 512):
                    t = srcp.tile([128, EC, 512], f8, name=f"{tag}{c}")
                    eng = (nc.sync, nc.gpsimd)[(q0 + c) % 2]
                    eng.dma_start(t, dram[c])
                    ts.append(t)
                return ts

            sub_t = load_src(subT_d, "sub", 0)
            load_w("sk")
            load_w("sv")
            load_w("q")
            scn_t = load_src(scnT_d, "scn", 1)

            # single-bank denominator PSUM: two independent accumulation
            # column-pairs (per-element has_written) alternate per subtile
            den2 = dpp.tile([128, 4], f32, name="den2")

            # ---------- emission helpers (shared PSUM rotation) ----------
            def proj_K(src, wc, bias, KT, conv_dve):
                """KT[p_lo, pc, k] = QS*(x @ Wk.T + b)[k, p].T"""
                for pc in range(EC):
                    for kn in range(SKV // 512):
                        ps = pp.tile([128, 512], f32, tag="ps512", bufs=3,
                                     name="ps_k")
                        for i in range(NPAIR):
                            nc.tensor.matmul(
                                ps,
                                wc[i][:, :, pc * 128:(pc + 1) * 128],
                                src[kn][:, 2 * i:2 * i + 2, :],
                                start=(i == 0), stop=(i == NPAIR - 1),
                                perf_mode=DR,
                            )
                        dst = KT[:, pc, kn * 512:(kn + 1) * 512]
                        if conv_dve:
                            nc.vector.tensor_scalar(
                                dst, ps, QS / WS, bias[:, pc:pc + 1],
                                op0=Alu.mult, op1=Alu.add)
                        else:
                            nc.scalar.activation(
                                dst, ps, Act.Identity,
                                bias=bias[:, pc:pc + 1], scale=QS / WS)

            def proj_V(src, wc, VT, vsum):
                """VT[k_lo, kc, p] = (x @ Wv.T)[k, p]  (bias folded out);
                also fills vsum[:, kc, 1] with the V row-sums."""
                for kc in range(EC):
                    st = src[kc // 4]
                    k0 = (kc % 4) * 128
                    acc = []
                    for po in range(PROJ // 512):
                        ps = pp.tile([128, 512], f32, tag="ps512", bufs=3,
                                     name="ps_v")
                        for i in range(NPAIR):
                            nc.tensor.matmul(
                                ps,
                                st[:, 2 * i:2 * i + 2, k0:k0 + 128],
                                wc[i][:, :, po * 512:(po + 1) * 512],
                                start=(i == 0), stop=(i == NPAIR - 1),
                                perf_mode=DR,
                            )
                        a = smp.tile([128, 1], f32, tag="sm", bufs=64,
                                     name="vacc")
                        nc.vector.tensor_scalar(
                            VT[:, kc, po * 512:(po + 1) * 512], ps, 1.0 / WS,
                            0.0, op0=Alu.mult, op1=Alu.add, accum_out=a)
                        acc.append(a)
                    nc.vector.tensor_add(vsum[:, kc, 1:2], acc[0], acc[1])

            def proj_Q(qc):
                ot = srcp.tile([128, EC, 512], f8, tag="ot", bufs=2,
                               name=f"ot{qc}")
                eng = (nc.sync, nc.gpsimd)[qc % 2]
                eng.dma_start(ot, objT_d[qc])
                for pc in range(EC):
                    ps = pp.tile([128, 512], f32, tag="ps512", bufs=3,
                                 name="ps_q")
                    for i in range(NPAIR):
                        nc.tensor.matmul(
                            ps,
                            wch["q"][i][:, :, pc * 128:(pc + 1) * 128],
                            ot[:, 2 * i:2 * i + 2, :],
                            start=(i == 0), stop=(i == NPAIR - 1),
                            perf_mode=DR,
                        )
                    dst = qt[:, pc, qc * 512:(qc + 1) * 512]
                    if pc % 2 < QDVE:
                        nc.vector.tensor_scalar(
                            dst, ps, QS / WS, bq_s[:, pc:pc + 1],
                            op0=Alu.mult, op1=Alu.add)
                    else:
                        nc.scalar.activation(
                            dst, ps, Act.Identity,
                            bias=bq_s[:, pc:pc + 1], scale=QS / WS)

            def scores_units(KT, qc, tag):
                et = etp.tile([128, EC, 512], f8, tag=tag, bufs=2, name=tag)

                def unit(kc):
                    def f():
                        ps = pp.tile([128, 512], f32, tag="ps512", bufs=3,
                                     name="sps")
                        for i in range(NPAIR):
                            nc.tensor.matmul(
                                ps,
                                KT[:, 2 * i:2 * i + 2,
                                   kc * 128:(kc + 1) * 128],
                                qt[:, 2 * i:2 * i + 2,
                                   qc * 512:(qc + 1) * 512],
                                start=(i == 0), stop=(i == NPAIR - 1),
                                perf_mode=DR,
                            )
                        nc.scalar.activation(et[:, kc, :], ps, Act.Exp,
                                             scale=SCALE / (QS * QS))
                    return f
                return et, [unit(kc) for kc in range(EC)]

            def scores(KT, qc, tag):
                et, units = scores_units(KT, qc, tag)
                for u in units:
                    u()
                return et

            def proj_Q_units(qc):
                ot = srcp.tile([128, EC, 512], f8, tag="ot", bufs=2,
                               name=f"otu{qc}")
                eng = (nc.sync, nc.gpsimd)[qc % 2]
                eng.dma_start(ot, objT_d[qc])

                def unit(pc):
                    def f():
                        ps = pp.tile([128, 512], f32, tag="ps512", bufs=3,
                                     name="ps_q")
                        for i in range(NPAIR):
                            nc.tensor.matmul(
                                ps,
                                wch["q"][i][:, :, pc * 128:(pc + 1) * 128],
                                ot[:, 2 * i:2 * i + 2, :],
                                start=(i == 0), stop=(i == NPAIR - 1),
                                perf_mode=DR,
                            )
                        dst = qt[:, pc, qc * 512:(qc + 1) * 512]
                        if pc % 2 < QDVE:
                            nc.vector.tensor_scalar(
                                dst, ps, QS / WS, bq_s[:, pc:pc + 1],
                                op0=Alu.mult, op1=Alu.add)
                        else:
                            nc.scalar.activation(
                                dst, ps, Act.Identity,
                                bias=bq_s[:, pc:pc + 1], scale=QS / WS)
                    return f
                return [unit(pc) for pc in range(EC)]

            ecnt = [0]

            def pv_block(et, VT, vsum, so_s, ob, out_d, qc, qs):
                """x' = d*obj + O assembled entirely in PSUM:
                PV matmuls + a diag(d) @ obj matmul; mean from precomputed
                row-sums; variance from one Square pass over the PSUM."""
                idx = qc * 4 + qs
                q0 = qs * 128
                c = ecnt[0]
                ecnt[0] += 1
                ops = opp.tile([128, PROJ], f32, tag="ops", bufs=2,
                               name="ops")
                dc = (c % 2) * 2
                den = den2[:, dc:dc + 2]
                # d and SumO in one tiny DR matmul group
                for i in range(NPAIR):
                    nc.tensor.matmul(
                        den, et[:, 2 * i:2 * i + 2, q0:q0 + 128],
                        vsum[:, 2 * i:2 * i + 2, :],
                        start=(i == 0), stop=(i == NPAIR - 1),
                        perf_mode=DR,
                    )
                # d, SumO to SBUF; build diag(d) while PV matmuls run
                ds = smp.tile([128, 2], f32, tag="sm", bufs=64, name="ds")
                nc.vector.tensor_copy(ds, den)
                d_col = ds[:, 0:1]
                D_t = epi.tile([128, 128], bf16, tag="D", bufs=2, name="D")
                nc.vector.tensor_scalar_mul(D_t, I_s, d_col)
                for i in range(NPAIR):
                    stat = et[:, 2 * i:2 * i + 2, q0:q0 + 128]
                    for po in range(PROJ // 512):
                        nc.tensor.matmul(
                            ops[:, po * 512:(po + 1) * 512],
                            stat,
                            VT[:, 2 * i:2 * i + 2, po * 512:(po + 1) * 512],
                            start=(i == 0), stop=False,
                            perf_mode=DR,
                        )
                # += diag(d) @ obj  (bf16, ordinary matmuls close the group)
                for po in range(PROJ // 512):
                    nc.tensor.matmul(
                        ops[:, po * 512:(po + 1) * 512],
                        D_t,
                        ob[:, po * 512:(po + 1) * 512],
                        start=False, stop=(po == 1),
                    )
                # ---- stats: mean from row-sums, variance from one pass ----
                epsd2 = smp.tile([128, 1], f32, tag="sm", bufs=64,
                                 name="epsd2")
                nc.vector.scalar_tensor_tensor(
                    epsd2, d_col, EPS, d_col, op0=Alu.mult, op1=Alu.mult)
                if c % BNMOD != 0:
                    mu_u = smp.tile([128, 1], f32, tag="sm", bufs=64,
                                    name="mu_u")
                    nc.vector.scalar_tensor_tensor(
                        mu_u, so_s[:, idx:idx + 1], d_col, ds[:, 1:2],
                        op0=Alu.mult, op1=Alu.add)
                nmu = smp.tile([128, 1], f32, tag="sm", bufs=64, name="nmu")
                var = smp.tile([128, 1], f32, tag="sm", bufs=64, name="var")
                if c % BNMOD == 0:
                    # DVE stats: bn_stats straight off the PSUM
                    bst = smp.tile([128, 2, 6], f32, tag="sm", bufs=64,
                                   name="bst")
                    nc.vector.bn_stats(bst[:, 0:1, :], ops[:, 0:512])
                    nc.vector.bn_stats(bst[:, 1:2, :], ops[:, 512:1024])
                    mv = smp.tile([128, 2], f32, tag="sm", bufs=64,
                                  name="mv")
                    nc.vector.bn_aggr(mv, bst)
                    nc.vector.tensor_scalar_mul(nmu, mv[:, 0:1], -1.0)
                    nc.vector.tensor_add(var, mv[:, 1:2], epsd2)
                else:
                    # ACT stats: Square+accum off the PSUM, mean from the
                    # precomputed row-sums
                    ssq = smp.tile([128, 1], f32, tag="sm", bufs=64,
                                   name="ssq")
                    sq = epi.tile([128, EMB], bf16, tag="sq", bufs=3,
                                  name="sq")
                    nc.scalar.activation(sq, ops, Act.Square, accum_out=ssq)
                    nc.vector.tensor_scalar_mul(nmu, mu_u, -1.0 / EMB)
                    msqu = smp.tile([128, 1], f32, tag="sm", bufs=64,
                                    name="msqu")
                    nc.vector.tensor_mul(msqu, mu_u, mu_u)
                    v1 = smp.tile([128, 1], f32, tag="sm", bufs=64,
                                  name="v1")
                    nc.vector.scalar_tensor_tensor(
                        v1, ssq, 1.0 / EMB, epsd2, op0=Alu.mult, op1=Alu.add)
                    nc.vector.scalar_tensor_tensor(
                        var, msqu, -1.0 / (EMB * EMB), v1, op0=Alu.mult,
                        op1=Alu.add)
                # rstd = exp(-0.5*ln(var)); nmr = -mu*rstd  (ACT-internal)
                lnv = smp.tile([128, 1], f32, tag="sm", bufs=64, name="lnv")
                nc.scalar.activation(lnv, var, Act.Ln)
                rstd = smp.tile([128, 1], f32, tag="sm", bufs=64,
                                name="rstd")
                nc.scalar.activation(rstd, lnv, Act.Exp, scale=-0.5)
                nmr = smp.tile([128, 1], f32, tag="sm", bufs=64, name="nmr")
                if NMRACT:
                    nc.scalar.activation(nmr, rstd, Act.Identity, scale=nmu)
                else:
                    nc.vector.tensor_mul(nmr, rstd, nmu)
                t = epi.tile([128, EMB], bf16, tag="t", bufs=4, name="t")
                nc.scalar.activation(t, ops, Act.Identity, bias=nmr,
                                     scale=rstd)
                if skip_gb:
                    # gamma==1, beta==0: t is the final output
                    nc.sync.dma_start(out_d[idx], t)
                else:
                    o1 = epi.tile([128, EMB], bf16, tag="o1", bufs=3,
                                  name="o1")
                    nc.vector.tensor_mul(o1, t, G_s)
                    o = epi.tile([128, EMB], bf16, tag="o", bufs=4, name="o")
                    nc.gpsimd.tensor_add(o, o1, B_s)
                    nc.sync.dma_start(out_d[idx], o)

            def fetch_obs(objr_d, qc):
                obs = []
                for qs in range(4):
                    ob = epi.tile([128, EMB], bf16, tag="ob", bufs=10,
                                  name="ob")
                    nc.sync.dma_start(ob, objr_d[qc * 4 + qs])
                    obs.append(ob)
                return obs

            def block(et, A, qc):
                VT, vsum, so_s, objr_d, out_d = A
                obs = fetch_obs(objr_d, qc)
                for qs in range(4):
                    pv_block(et, VT, vsum, so_s, obs[qs], out_d, qc, qs)

            def blockpair(etA, argsA, qcA, etB, argsB, qcB,
                          units=()):
                units = list(units)
                per = (len(units) + 7) // 8 if units else 0
                ui = 0
                obsA = fetch_obs(argsA[3], qcA)
                obsB = fetch_obs(argsB[3], qcB)
                for qs in range(4):
                    for eb, args, obs, qc in ((etA, argsA, obsA, qcA),
                                              (etB, argsB, obsB, qcB)):
                        for _ in range(per):
                            if ui < len(units):
                                units[ui]()
                                ui += 1
                        pv_block(eb, args[0], args[1], args[2], obs[qs],
                                 args[4], qc, qs)
                while ui < len(units):
                    units[ui]()
                    ui += 1

            A1 = (vt1, vsum1, so1_s, objr1_d, I1_d)
            A2 = (vt2, vsum2, so2_s, objr2_d, I2_d)

            # ---------- woven schedule ----------
            proj_K(sub_t, wch["sk"], bsk_s, kt1, conv_dve=K1DVE)
            proj_V(sub_t, wch["sv"], vt1, vsum1)
            proj_Q(0)
            load_w("ek")
            load_w("ev")
            e1_0 = scores(kt1, 0, "et1")
            proj_K(scn_t, wch["ek"], bek_s, kt2, conv_dve=K2DVE)
            block(e1_0, A1, 0)
            proj_Q(1)
            e2_0 = scores(kt2, 0, "et2")
            e1_1 = scores(kt1, 1, "et1")
            proj_V(scn_t, wch["ev"], vt2, vsum2)
            uQ2 = proj_Q_units(2)
            e2_1_et, u21 = scores_units(kt2, 1, "et2")
            e1_2_et, u12 = scores_units(kt1, 2, "et1")
            blockpair(e2_0, A2, 0, e1_1, A1, 1, uQ2 + u21 + u12)
            uQ3 = proj_Q_units(3)
            e2_2_et, u22 = scores_units(kt2, 2, "et2")
            e1_3_et, u13 = scores_units(kt1, 3, "et1")
            blockpair(e2_1_et, A2, 1, e1_2_et, A1, 2, uQ3 + u22 + u13)
            e2_3_et, u23 = scores_units(kt2, 3, "et2")
            blockpair(e2_2_et, A2, 2, e1_3_et, A1, 3, u23)
            block(e2_3_et, A2, 3)

    nc.compile()
    return nc


def _prep_in_maps(inputs):
    f8 = ml_dtypes.float8_e4m3
    bf = ml_dtypes.bfloat16
    f = lambda a: np.asarray(a, dtype=np.float32)
    obj = f(inputs["obj"])
    sub = f(inputs["sub"])
    scene = f(inputs["scene"])

    def chunk_xT(xT, width):
        # xT [EMB, S] -> [S//width, 128, 8, width] fp8
        S = xT.shape[1]
        t = xT.reshape(EC, 128, S).transpose(1, 0, 2)  # [128, 8, S]
        t = t.reshape(128, EC, S // width, width).transpose(2, 0, 1, 3)
        return np.ascontiguousarray(t).astype(f8)

    shared = {}
    for n in ["q", "sk", "ek", "sv", "ev"]:
        wt = f(inputs[f"W_{n}"]).T * WS  # [EMB, PROJ]
        t = wt.reshape(EC, 128, PROJ).transpose(1, 0, 2)  # [128, 8, PROJ]
        t = t.reshape(128, NPAIR, 2, PROJ).transpose(1, 0, 2, 3)
        shared[f"W{n}T"] = np.ascontiguousarray(t).astype(f8)
    for key, n in [("bq", "q"), ("bsk", "sk"), ("bek", "ek")]:
        shared[key] = np.ascontiguousarray(
            (f(inputs[f"b_{n}"]) * QS).reshape(EC, 128).T)
    shared["G"] = np.ascontiguousarray(
        np.broadcast_to(f(inputs["ln_g"]), (128, EMB))).astype(bf)
    shared["Bb"] = np.ascontiguousarray(
        np.broadcast_to(f(inputs["ln_b"]), (128, EMB))).astype(bf)
    b_sv = f(inputs["b_sv"])
    b_ev = f(inputs["b_ev"])
    shared["ident"] = np.eye(128, dtype=np.float32).astype(bf)

    in_maps = []
    for b in range(NCORES):
        m = dict(shared)
        m["objT"] = chunk_xT(obj[b].T, 512)
        m["subT"] = chunk_xT(sub[b].T, 512)
        m["scnT"] = chunk_xT(scene[b].T, 512)
        for key, bias in (("objr1", b_sv), ("objr2", b_ev)):
            r = (obj[b] + bias[None, :]).reshape(
                SQ // 128, 128, EMB).astype(bf)
            m[key] = r
            # per-row sums of the bf16-rounded residual, [128, 16]
            m[key + "s"] = np.ascontiguousarray(
                r.astype(np.float32).sum(axis=2).T)
        in_maps.append(m)
    return in_maps


def kernel(**inputs):
    global LAST_RESULTS
    from concourse import bass_utils

    g = np.asarray(inputs["ln_g"], dtype=np.float32)
    b = np.asarray(inputs["ln_b"], dtype=np.float32)
    skip_gb = bool(np.all(g == 1.0) and np.all(b == 0.0))
    key = ("nc", skip_gb)
    if key not in _CACHE:
        _CACHE[key] = _build(skip_gb=skip_gb)
    nc = _CACHE[key]
    in_maps = _prep_in_maps(inputs)
    res = bass_utils.run_bass_kernel_spmd(
        nc, in_maps, core_ids=list(range(NCORES)))
    LAST_RESULTS = res
    I1 = np.stack([
        res.results[c]["I1"].astype(np.float32).reshape(SQ, EMB)
        for c in range(NCORES)])
    I2 = np.stack([
        res.results[c]["I2"].astype(np.float32).reshape(SQ, EMB)
        for c in range(NCORES)])
    return I1, I2
